# revision 1
# baseline (speedup 1.0000x reference)
"""Single-directional Chamfer distance (pytorch3d semantics) on 8 trn2 cores.

loss = mean_b mean_i min_j ||x_bi - y_bj||^2   with x = v_pred, y = v.

Sharding: batch B=8 across the 8 cores, one point-cloud pair per core.

Per-core algorithm (all-pairs):
  d2[i,j] = xsq_i - 2 x_i.y_j + ysq_j
          = xsq_i - 2 * out[i,j],   out[i,j] = x_i.y_j - ysq_j/2
  out computed by the PE as a K=4 matmul over augmented coordinates:
     lhsT rows = (1, x0, x1, x2)        [4, 128]  stationary, per query block
     rhs  rows = (-ysq/2, y0, y1, y2)   [4, 512]  moving
  min_j d2 = xsq_i - 2 * max_j out[i,j]  -> DVE reduce_max over PSUM tiles.
  loss_core = (sum_i xsq_i - 2 * sum_i max_j out[i,j]) / N

Raw bass implementation (explicit semaphores): the ISA allows only one
sync-wait per instruction, so the PE/DVE ping-pong over two 4-bank PSUM
buffers is hand-synchronized with one semaphore each way.

The augmented [4, N] operands are marshalled on the host (transpose, a ones
row, and the -|y|^2/2 norm row); all O(N^2) work runs on device.
"""

import os

import numpy as np

import concourse.bass as bass
import concourse.mybir as mybir
from concourse.bass_utils import run_bass_kernel_spmd

F32 = mybir.dt.float32
N = 16384
NCORES = 8

TBS = 2048           # targets per DVE reduce (4 PSUM banks)
NTB = N // TBS       # 8
NQB = N // 128       # 128 query blocks

_BUILD_CACHE = {}


def _build():
    nc = bass.Bass()
    x4 = nc.dram_tensor("x4", [4, N], F32, kind="ExternalInput")
    y4 = nc.dram_tensor("y4", [4, N], F32, kind="ExternalInput")
    xn = nc.dram_tensor("xn", [N, 3], F32, kind="ExternalInput")
    out = nc.dram_tensor("out", [2, 128], F32, kind="ExternalOutput")

    AX = mybir.AxisListType
    OP = mybir.AluOpType
    NT = NQB * NTB   # 1024 psum tiles

    with (
        nc.sbuf_tensor([4, N], F32) as lhsT,
        nc.sbuf_tensor([4, N], F32) as rhs,
        nc.sbuf_tensor([128, 384], F32) as x_nat,
        nc.sbuf_tensor([128, 384], F32) as xsq,
        nc.sbuf_tensor([128, NTB], F32) as mcols,
        nc.sbuf_tensor([128, NQB], F32) as m_grid,
        nc.sbuf_tensor([128, 1], F32) as s_m,
        nc.sbuf_tensor([128, 1], F32) as s_x,
        nc.psum_tensor([128, TBS], F32) as psA,
        nc.psum_tensor([128, TBS], F32) as psB,
        nc.semaphore() as dma_sem,
        nc.semaphore() as mm_sem,
        nc.semaphore() as red_sem,
        nc.semaphore() as dve_done,
        nc.semaphore() as chain_sem,
        nc.semaphore() as qb_sem,
        nc.Block() as block,
    ):

        @block.sync
        def _(sync):
            sync.dma_start(lhsT[:, :], x4[:, :]).then_inc(dma_sem, 16)
            sync.dma_start(rhs[:, :], y4[:, :]).then_inc(dma_sem, 16)
            sync.dma_start(
                x_nat[:, :], xn[:, :].rearrange("(p q) c -> p (q c)", p=128)
            ).then_inc(dma_sem, 16)
            sync.wait_ge(dve_done, 1)
            sync.dma_start(out[0:1, :].rearrange("a b -> b a"), s_m[:, :]).then_inc(dma_sem, 16)
            sync.dma_start(out[1:2, :].rearrange("a b -> b a"), s_x[:, :]).then_inc(dma_sem, 16)

        @block.tensor
        def _(tensor):
            tensor.wait_ge(dma_sem, 48)
            for tidx in range(NT):
                qb, tb = divmod(tidx, NTB)
                ps = psA if tidx % 2 == 0 else psB
                if tidx >= 2:
                    # wait until the reduce of tile tidx-2 released this
                    # psum buffer (red_sem counts finished reduces)
                    tensor.wait_ge(red_sem, tidx - 1)
                lw = lhsT[:, qb * 128 : (qb + 1) * 128]
                for k in range(TBS // 512):
                    c0 = tb * TBS + k * 512
                    mm = nc.tensor.matmul(
                        ps[:, k * 512 : (k + 1) * 512],
                        lw,
                        rhs[:, c0 : c0 + 512],
                        start=True,
                        stop=True,
                    )
                mm.then_inc(mm_sem, 1)

        @block.vector
        def _(vector):
            vector.wait_ge(dma_sem, 48)
            nc.vector.tensor_mul(xsq[:, :], x_nat[:, :], x_nat[:, :]).then_inc(
                chain_sem, 1
            )
            vector.wait_ge(chain_sem, 1)
            nc.vector.tensor_reduce(s_x[:, :], xsq[:, :], axis=AX.X, op=OP.add)
            for tidx in range(NT):
                qb, tb = divmod(tidx, NTB)
                ps = psA if tidx % 2 == 0 else psB
                vector.wait_ge(mm_sem, tidx + 1)
                if tb == 0 and qb > 0:
                    # WAR: previous qb's second-stage read of mcols
                    vector.wait_ge(qb_sem, qb)
                nc.vector.tensor_reduce(
                    mcols[:, tb : tb + 1], ps[:, :], axis=AX.X, op=OP.max
                ).then_inc(red_sem, 1)
                if tb == NTB - 1:
                    # red_sem counts completed first-stage reduces: waiting
                    # >= tidx+1 orders this read of mcols after all 8 writes
                    vector.wait_ge(red_sem, tidx + 1)
                    nc.vector.tensor_reduce(
                        m_grid[:, qb : qb + 1], mcols[:, :], axis=AX.X, op=OP.max
                    ).then_inc(qb_sem, 1)
            vector.wait_ge(qb_sem, NQB)
            nc.vector.tensor_reduce(
                s_m[:, :], m_grid[:, :], axis=AX.X, op=OP.add
            ).then_inc(dve_done, 1)

    return nc


def _marshal(v: np.ndarray, v_pred: np.ndarray):
    """Host-side operand marshalling: transposes, a ones row and the
    -|y|^2/2 norm row (augmented coordinates for the distance matmul)."""
    in_maps = []
    for b in range(NCORES):
        x4 = np.empty((4, N), np.float32)
        x4[0, :] = 1.0
        x4[1:4, :] = v_pred[b].T
        y4 = np.empty((4, N), np.float32)
        y4[0, :] = -0.5 * (v[b].astype(np.float64) ** 2).sum(axis=1)
        y4[1:4, :] = v[b].T
        in_maps.append(
            {"x4": x4, "y4": y4, "xn": np.ascontiguousarray(v_pred[b])}
        )
    return in_maps


def kernel(v: np.ndarray, v_pred: np.ndarray) -> np.ndarray:
    v = np.ascontiguousarray(np.asarray(v, dtype=np.float32))
    v_pred = np.ascontiguousarray(np.asarray(v_pred, dtype=np.float32))
    assert v.shape == (NCORES, N, 3) and v_pred.shape == (NCORES, N, 3)

    if "k" not in _BUILD_CACHE:
        _BUILD_CACHE["k"] = _build()
    nc = _BUILD_CACHE["k"]

    in_maps = _marshal(v, v_pred)
    res = run_bass_kernel_spmd(
        nc,
        in_maps,
        core_ids=list(range(NCORES)),
        trace=bool(int(os.environ.get("BASS_TRACE_KERNEL", "0"))),
    )
    if res.exec_time_ns is not None:
        print(f"HW exec time: {res.exec_time_ns} ns")

    per_core = []
    for r in res.results:
        o = np.asarray(r["out"], dtype=np.float64)
        s_m = o[0, :].sum()
        s_x = o[1, :].sum()
        per_core.append((s_x - 2.0 * s_m) / N)
    loss = np.float32(np.mean(per_core))
    return np.array(loss, dtype=np.float32)

